# revision 20
# baseline (speedup 1.0000x reference)
"""MHA forward (dense transformer block) for TRN2, 8 NeuronCores.

Sharding: core c handles batch b = c // 4 and head-group g = c % 4
(4 heads of 64 dims = 256 hidden columns).  Wq/Wkv are sharded
column-wise, Wo row-wise; each core produces a partial [2048, 1024]
bf16 output which the host sums per batch (+ bo).

Per-core pipeline (all matmul operands bf16; host pre-casts q/W and
pre-transposes q so no PE transposes are needed on device):
  1. DMA qT [D on partitions, S] (bf16) in 512-col span chunks.
  2. Q^T/K^T projections packed 2 heads per 128 partitions; V projection
     in natural layout augmented with a ones column per head (bias trick)
     so the attn@V matmul also produces the softmax denominator.
  3. Scores S^T = K^T.T @ Q^T in 512-wide q-spans; exp mostly on ACT
     (scale folded in), a fraction on DVE via a Schraudolph int16 affine
     (bitcast to bf16) to widen the PSUM drain; probs bf16.
  4. attn@V: lhsT = V' ([V_h | 64 ones cols], M=128), rhs = probs ->
     O^T in PSUM rows 0-63 with the softmax denominator broadcast into
     rows 64-127 by the static ones columns.
  5. Normalize O^T rows by 1/denom (DVE reciprocal + multiply), bf16.
  6. Output projection (bf16) -> partial (bf16) -> DRAM.

The emitter keeps the PE stream continuous (HAM throttling punishes PE
idle gaps): between score groups it interleaves "filler" matmul units
(V/K/Q projection blocks, previous span's attn@V chains, outproj) from a
FIFO of generators, so the PE never waits on the exp drain of the score
PSUM ring.
"""

import sys

sys.path.insert(0, "/opt/trn_rl_repo")

from collections import deque

import numpy as np
import ml_dtypes

import concourse.bass as bass
from concourse import bacc
import concourse.mybir as mybir
import concourse.tile as tile
from concourse.bass_utils import run_bass_kernel_spmd
from concourse.masks import make_identity

F32 = mybir.dt.float32
I16 = mybir.dt.int16
BF16 = mybir.dt.bfloat16
AF = mybir.ActivationFunctionType
ALU = mybir.AluOpType

S = 2048          # sequence length per batch
D = 1024          # model dim
DH = 64           # head dim
NH = 4            # heads per core
GH = NH * DH      # 256 hidden cols per core
KC = D // 128     # 8 contraction chunks of 128
ST = S // 128     # 16 sequence blocks of 128
SPAN = 512        # q-span processed per scores/exp/attnV block
NSP = S // SPAN   # 4 spans
SCALE = DH ** -0.5

# Schraudolph exp on DVE/GpSimd: i16 = trunc(s*C1 + C2); bits viewed as
# bf16 approximate exp(s*SCALE) within ~3.6% max rel err.  2 of every 3
# score tiles go through the approximation (DVE + GpSimd), 1 of 3 through
# exact exp on ACT.
C1 = SCALE * np.log2(np.e) * 128.0
C2 = (127.0 - 0.045) * 128.0 + 0.5

FILL_NS = 14000   # PE filler budget between score bursts (ns)

N_CORES = 8


def _build_nc(reps=1):
    nc = bacc.Bacc("TRN2", target_bir_lowering=False)

    qb = nc.declare_dram_parameter("qt", [D, S], BF16, isOutput=False)
    wq = nc.declare_dram_parameter("wq", [D, GH], BF16, isOutput=False)
    wk = nc.declare_dram_parameter("wk", [D, GH], BF16, isOutput=False)
    wv = nc.declare_dram_parameter("wv", [D, GH], BF16, isOutput=False)
    bqk = nc.declare_dram_parameter("bqk", [128, 4], F32, isOutput=False)
    bvb = nc.declare_dram_parameter("bvb", [128, GH], F32, isOutput=False)
    wo = nc.declare_dram_parameter("wo", [GH, D], BF16, isOutput=False)
    out = nc.declare_dram_parameter("partial", [S, D], BF16, isOutput=True)

    with tile.TileContext(nc) as tc:
        with (
            tc.tile_pool(name="wsing", bufs=1) as wsing,
            tc.tile_pool(name="big", bufs=8) as big,
            tc.tile_pool(name="ptp", bufs=6) as ptp,
            tc.tile_pool(name="qk", bufs=6) as qk,
            tc.tile_pool(name="small", bufs=1) as small,
            tc.tile_pool(name="ostage", bufs=4) as ostage,
            tc.tile_pool(name="pmm", bufs=2, space="PSUM") as pmm,
            tc.tile_pool(name="pov", bufs=2, space="PSUM") as pov,
            tc.tile_pool(name="pst", bufs=4, space="PSUM") as pst,
        ):
            # Weight DMAs ride the ACT hwdge / gpsimd swdge queues so the
            # SP queue is free for the qT span chunks the projections need
            # first.
            wk_t = wsing.tile([128, KC, GH], BF16)
            nc.gpsimd.dma_start(out=wk_t, in_=wk[:, :].rearrange("(kc p) f -> p kc f", p=128))
            wq_t = wsing.tile([128, KC, GH], BF16)
            nc.gpsimd.dma_start(out=wq_t, in_=wq[:, :].rearrange("(kc p) f -> p kc f", p=128))
            wv_t = wsing.tile([128, KC, GH], BF16)
            nc.gpsimd.dma_start(out=wv_t, in_=wv[:, :].rearrange("(kc p) f -> p kc f", p=128))
            wo_t = wsing.tile([128, 2, D], BF16)
            nc.gpsimd.dma_start(out=wo_t, in_=wo[:, :].rearrange("(c p) f -> p c f", p=128))
            bqk_t = wsing.tile([128, 4], F32)
            nc.scalar.dma_start(out=bqk_t, in_=bqk[:, :])
            bvb_t = wsing.tile([128, GH], F32)
            nc.scalar.dma_start(out=bvb_t, in_=bvb[:, :])
            # Persistent V' [128, kb, 4 heads x (64 V | 64 ones)]: the ones
            # columns are memset once; attn@V's lhsT M=128 then broadcasts
            # the softmax denominator across PSUM partitions 64-127 free.
            vpr_t = wsing.tile([128, ST, 512], BF16, name="vpr_t")
            nc.vector.memset(
                vpr_t[:, :, :].rearrange("p kb (h c) -> p (kb h) c", c=128)[:, :, 64:128],
                1.0,
            )

            consts = (qb, out, wq_t, wk_t, wv_t, wo_t, bqk_t, bvb_t, vpr_t)
            pools = (big, ptp, qk, small, ostage, pmm, pov, pst)

            def emit_body():
                _emit_body(nc, consts, pools)

            if reps == 1:
                emit_body()
            else:
                # For_i ends each iteration with an all-engine barrier +
                # semaphore reset, so loop iterations cannot overlap.  Unroll
                # the body so body k+1's qT prefetch DMAs overlap body k's
                # compute tail, amortizing the startup/tail latency.
                U = 4 if reps % 4 == 0 else (2 if reps % 2 == 0 else 1)
                with tc.For_i(0, reps // U, 1):
                    for _ in range(U):
                        emit_body()

    nc.compile()
    return nc


def _emit_body(nc, consts, pools):
    (qb, out, wq_t, wk_t, wv_t, wo_t, bqk_t, bvb_t, vpr_t) = consts
    (big, ptp, qk, small, ostage, pmm, pov, pst) = pools

    # ---- filler machinery: FIFO of (span_tag, generator) ----
    fill = deque()

    def push(tag, gen):
        fill.append((tag, gen))

    def run_fill(budget):
        while budget > 0 and fill:
            tag, gen = fill[0]
            try:
                budget -= next(gen)
            except StopIteration:
                fill.popleft()

    def flush_through(tag_max):
        while fill and fill[0][0] <= tag_max:
            tag, gen = fill[0]
            for _ in gen:
                pass
            fill.popleft()

    def flush_all():
        flush_through(1 << 30)

    # ---- qT spans DMA'd directly from DRAM (host pre-transposed) ----
    # Span-major chunk order so the span-0 projections can start after
    # ~1MB of DMA instead of the full 4MB.
    qT = [big.tile([128, S], BF16, tag="big", name=f"qT{j}") for j in range(KC)]

    def prefetch_qt():
        for sp4 in range(4):
            for j in range(KC):
                eng = nc.sync if j % 2 == 0 else nc.scalar
                eng.dma_start(
                    out=qT[j][:, sp4 * 512:(sp4 + 1) * 512],
                    in_=qb[j * 128:(j + 1) * 128, sp4 * 512:(sp4 + 1) * 512],
                )

    # ---- projections ----
    QT = [None, None]
    KT = [None, None]

    def qk_chain(pair, w_t, bias_col, dst, sp4):
        ps = pmm.tile([128, 512], F32, tag="pmm")
        for k in range(KC):
            nc.tensor.matmul(
                ps,
                w_t[:, k, pair * 128:(pair + 1) * 128],
                qT[k][:, sp4 * 512:(sp4 + 1) * 512],
                start=(k == 0),
                stop=(k == KC - 1),
            )
        nc.vector.tensor_scalar_add(
            dst[:, sp4 * 512:(sp4 + 1) * 512],
            ps,
            bqk_t[:, bias_col:bias_col + 1],
        )

    def chain_gen(pair, w_t, bias_col, dst, sp4):
        qk_chain(pair, w_t, bias_col, dst, sp4)
        yield 1700

    def v_gen(blocks):
        for sb in blocks:
            ps = pmm.tile([128, 512], F32, tag="pmm")
            for k in range(KC):
                nc.tensor.matmul(
                    ps[:, :GH],
                    qT[k][:, sb * 128:(sb + 1) * 128],
                    wv_t[:, k, :],
                    start=(k == 0),
                    stop=(k == KC - 1),
                )
            nc.vector.tensor_add(
                vpr_t[:, sb, :].rearrange("p (h c) -> p h c", c=128)[:, :, 0:64],
                ps[:, :GH].rearrange("p (h c) -> p h c", c=64),
                bvb_t.rearrange("p (h c) -> p h c", c=64),
            )
            yield 870

    OT = [qk.tile([128, S], BF16, tag="qk", name=f"OT{c}") for c in range(2)]

    # ---- attention stages ----
    def sx_burst(pair, sp):
        """All 32 score matmuls of one (head-pair, q-span) back-to-back in
        64x128 row-tile mode (no mode switches inside).  Scores land in PSUM
        as bf16 (halves the score PSUM footprint -> 8-deep ring in 4 banks,
        and enables the DVE 2x 16-bit path) and the exp drains alternate
        ACT (exact exp) / DVE (Schraudolph int16) at a ~2:3 ratio matched to
        their per-tile costs."""
        q0 = sp * SPAN
        pt = [ptp.tile([128, ST, SPAN], BF16, tag="pt", name=f"pt{pair}_{sp}_{h}")
              for h in range(2)]
        i = 0
        for kb in range(ST):
            for h in range(2):
                ps = pst.tile([128, SPAN], F32, tag="st", name=f"st{kb}_{h}")
                nc.tensor.matmul(
                    ps,
                    KT[pair][h * 64:(h + 1) * 64, kb * 128:(kb + 1) * 128],
                    QT[pair][h * 64:(h + 1) * 64, q0:q0 + SPAN],
                    start=True,
                    stop=True,
                )
                pt_sl = pt[h][:, kb, :]
                kind = i % 2
                i += 1
                if kind == 0:
                    nc.scalar.activation(pt_sl, ps, AF.Exp, scale=SCALE)
                else:
                    nc.vector.tensor_scalar(
                        pt_sl.bitcast(I16), ps, C1, C2, ALU.mult, ALU.add)
        return pt

    def av_gen(pair, sp, pt):
        """attn@V + normalize, as ~0.9 µs units."""
        for h in range(2):
            hh = pair * 2 + h
            ov = pov.tile([128, SPAN], F32, tag="ov")
            for kq in range(4):
                for kb in range(kq * 4, kq * 4 + 4):
                    nc.tensor.matmul(
                        ov,
                        vpr_t[:, kb, hh * 128:(hh + 1) * 128],
                        pt[h][:, kb, :],
                        start=(kb == 0),
                        stop=(kb == ST - 1),
                    )
                yield 870
            rdb = small.tile([64, SPAN], F32, tag="rdb")
            nc.vector.reciprocal(rdb, ov[64:128, :])
            nc.vector.tensor_mul(
                OT[pair][h * 64:(h + 1) * 64, sp * SPAN:(sp + 1) * SPAN],
                ov[0:DH, :],
                rdb,
            )
            yield 400

    def outproj_gen(sbs, copy_on_act=False):
        for sb in sbs:
            o_tile = ostage.tile([128, D], BF16, tag="ostage")
            for n in range(2):
                ps = pmm.tile([128, 512], F32, tag="pmm")
                for c in range(2):
                    nc.tensor.matmul(
                        ps,
                        OT[c][:, sb * 128:(sb + 1) * 128],
                        wo_t[:, c, n * 512:(n + 1) * 512],
                        start=(c == 0),
                        stop=(c == 1),
                    )
                if copy_on_act:
                    nc.scalar.copy(o_tile[:, n * 512:(n + 1) * 512], ps)
                else:
                    nc.vector.tensor_copy(o_tile[:, n * 512:(n + 1) * 512], ps)
            nc.gpsimd.dma_start(out=out[sb * 128:(sb + 1) * 128, :], in_=o_tile)
            yield 900

    # ---- emission: score bursts with 128x128-mode filler blocks between ----
    def proj_tile(pair, name):
        t = qk.tile([128, S], BF16, tag="qk", name=name)
        return t

    KT[0] = proj_tile(0, "ktp0")
    QT[0] = proj_tile(0, "qtp0")
    KT[1] = proj_tile(1, "ktp1")
    QT[1] = proj_tile(1, "qtp1")
    prefetch_qt()
    # Full K projection for pair 0 (burst (0,0) reads all key blocks) plus
    # the span-0 Q projection.
    for sp4 in range(4):
        qk_chain(0, wk_t, 2, KT[0], sp4)
    qk_chain(0, wq_t, 0, QT[0], 0)

    # Filler supply for the pair-0 spans: V projection, then pair-1 K/Q.
    push(0, v_gen(range(ST)))
    for sp4 in range(4):
        push(1, chain_gen(1, wk_t, 3, KT[1], sp4))
    for sp4 in range(4):
        push(1, chain_gen(1, wq_t, 1, QT[1], sp4))

    order = [(0, sp) for sp in range(NSP)] + [(1, sp) for sp in range(NSP)]
    for idx, (pair, sp) in enumerate(order):
        if pair == 0 and sp > 0:
            qk_chain(0, wq_t, 0, QT[0], sp)
        flush_through(idx - 3)
        pt = sx_burst(pair, sp)
        push(idx, av_gen(pair, sp, pt))
        if pair == 1:
            push(idx, outproj_gen(range(sp * 4, sp * 4 + 4), copy_on_act=(sp >= 2)))
        run_fill(FILL_NS)
    flush_all()


_NC_CACHE = {}


def _get_nc(reps=1):
    if reps not in _NC_CACHE:
        _NC_CACHE[reps] = _build_nc(reps)
    return _NC_CACHE[reps]


def _shard_inputs(q, Wq, bq, Wkv, bkv, Wo, bo):
    q = np.asarray(q, dtype=np.float32)
    Wq = np.asarray(Wq, dtype=np.float32)
    bq = np.asarray(bq, dtype=np.float32)
    Wkv = np.asarray(Wkv, dtype=np.float32)
    bkv = np.asarray(bkv, dtype=np.float32)
    Wo = np.asarray(Wo, dtype=np.float32)

    HID = D  # 1024 total hidden
    in_maps = []
    for c in range(N_CORES):
        b, g = divmod(c, 4)
        lo = g * GH
        wk_s = Wkv[:, lo:lo + GH]
        wv_s = Wkv[:, HID + lo:HID + lo + GH]
        bq_s = bq[lo:lo + GH]
        bk_s = bkv[lo:lo + GH]
        bv_s = bkv[HID + lo:HID + lo + GH]

        bqk_pp = np.stack(
            [bq_s[0:128], bq_s[128:256], bk_s[0:128], bk_s[128:256]], axis=1
        )

        in_maps.append({
            "qt": np.ascontiguousarray(q[b].T).astype(ml_dtypes.bfloat16),
            "wq": np.ascontiguousarray(Wq[:, lo:lo + GH]).astype(ml_dtypes.bfloat16),
            "wk": np.ascontiguousarray(wk_s).astype(ml_dtypes.bfloat16),
            "wv": np.ascontiguousarray(wv_s).astype(ml_dtypes.bfloat16),
            "bqk": np.ascontiguousarray(bqk_pp),
            "bvb": np.broadcast_to(bv_s, (128, GH)).copy(),
            "wo": np.ascontiguousarray(Wo[lo:lo + GH, :]).astype(ml_dtypes.bfloat16),
        })
    return in_maps


def _gather(results, bo):
    bo = np.asarray(bo, dtype=np.float32)
    out = np.empty((2, S, D), dtype=np.float32)
    for b in range(2):
        acc = results[4 * b]["partial"].astype(np.float32)
        for g in range(1, 4):
            acc = acc + results[4 * b + g]["partial"].astype(np.float32)
        out[b] = acc + bo
    return out


_RUNNER_CACHE = {}


def _make_runner(reps=1):
    """Build (once) a reusable jitted SPMD callable for the given rep count.

    Re-jitting per call loads a second copy of the NEFF and has been seen to
    wedge the exec unit, so the jitted executable is cached per process.
    """
    if reps in _RUNNER_CACHE:
        return _RUNNER_CACHE[reps]

    import jax
    from jax.sharding import Mesh, PartitionSpec
    from jax.experimental.shard_map import shard_map
    from concourse import bass2jax

    nc = _get_nc(reps)
    bass2jax.install_neuronx_cc_hook()
    partition_name = nc.partition_id_tensor.name if nc.partition_id_tensor else None
    in_names, out_names, out_avals, zero_outs = [], [], [], []
    for alloc in nc.m.functions[0].allocations:
        if not isinstance(alloc, mybir.MemoryLocationSet):
            continue
        name = alloc.memorylocations[0].name
        if alloc.kind == "ExternalInput":
            if name != partition_name:
                in_names.append(name)
        elif alloc.kind == "ExternalOutput":
            out_names.append(name)
            shape = tuple(alloc.tensor_shape)
            dtype = mybir.dt.np(alloc.dtype)
            out_avals.append(jax.core.ShapedArray(shape, dtype))
            zero_outs.append(np.zeros(shape, dtype))
    n_params = len(in_names)
    n_outs = len(out_avals)
    in_names.extend(out_names)
    if partition_name:
        in_names.append(partition_name)

    def _body(*args):
        operands = list(args)
        if partition_name:
            operands.append(bass2jax.partition_id_tensor())
        return tuple(bass2jax._bass_exec_p.bind(
            *operands,
            out_avals=tuple(out_avals),
            in_names=tuple(in_names),
            out_names=tuple(out_names),
            lowering_input_output_aliases=(),
            sim_require_finite=True,
            sim_require_nnan=True,
            nc=nc,
        ))

    devices = jax.devices()[:N_CORES]
    mesh = Mesh(np.asarray(devices), ("core",))
    donate = tuple(range(n_params, n_params + n_outs))
    sharded = jax.jit(
        shard_map(_body, mesh=mesh,
                  in_specs=(PartitionSpec("core"),) * (n_params + n_outs),
                  out_specs=(PartitionSpec("core"),) * len(out_names),
                  check_rep=False),
        donate_argnums=donate, keep_unused=True)

    def run(in_maps):
        per_core = [[np.asarray(m[nm]) for nm in in_names[:n_params]]
                    for m in in_maps]
        concat_in = [np.concatenate([per_core[c][i] for c in range(N_CORES)],
                                    axis=0) for i in range(n_params)]
        zo = [np.concatenate([z] * N_CORES, axis=0) for z in zero_outs]
        outs = sharded(*concat_in, *zo)
        outs = [np.asarray(o) for o in outs]
        per_core_res = []
        for c in range(N_CORES):
            per_core_res.append({
                name: np.split(outs[i], N_CORES, axis=0)[c]
                for i, name in enumerate(out_names)
            })
        return per_core_res

    _RUNNER_CACHE[reps] = run
    return run


def _run(inputs, reps=1):
    run = _make_runner(reps)
    in_maps = _shard_inputs(**inputs)
    results = run(in_maps)
    out = _gather(results, inputs["bo"])
    return out, results


def kernel(q, Wq, bq, Wkv, bkv, Wo, bo):
    out, _ = _run(dict(q=q, Wq=Wq, bq=bq, Wkv=Wkv, bkv=bkv, Wo=Wo, bo=bo))
    return out



# revision 23
# speedup vs baseline: 1.0727x; 1.0727x over previous
"""MHA forward (dense transformer block) for TRN2, 8 NeuronCores.

Sharding: core c handles batch b = c // 4 and head-group g = c % 4
(4 heads of 64 dims = 256 hidden columns).  Wq/Wkv are sharded
column-wise, Wo row-wise; each core produces a partial [2048, 1024]
bf16 output which the host sums per batch (+ bo).

Per-core pipeline (all matmul operands bf16; host pre-casts q/W and
pre-transposes q so no PE transposes are needed on device):
  1. DMA qT [D on partitions, S] (bf16) in 512-col span chunks.
  2. Q^T/K^T projections packed 2 heads per 128 partitions; V projection
     in natural layout augmented with a ones column per head (bias trick)
     so the attn@V matmul also produces the softmax denominator.
  3. Scores S^T = K^T.T @ Q^T in 512-wide q-spans; exp mostly on ACT
     (scale folded in), a fraction on DVE via a Schraudolph int16 affine
     (bitcast to bf16) to widen the PSUM drain; probs bf16.
  4. attn@V: lhsT = V' ([V_h | 64 ones cols], M=128), rhs = probs ->
     O^T in PSUM rows 0-63 with the softmax denominator broadcast into
     rows 64-127 by the static ones columns.
  5. Normalize O^T rows by 1/denom (DVE reciprocal + multiply), bf16.
  6. Output projection (bf16) -> partial (bf16) -> DRAM.

The emitter keeps the PE stream continuous (HAM throttling punishes PE
idle gaps): between score groups it interleaves "filler" matmul units
(V/K/Q projection blocks, previous span's attn@V chains, outproj) from a
FIFO of generators, so the PE never waits on the exp drain of the score
PSUM ring.
"""

import sys

sys.path.insert(0, "/opt/trn_rl_repo")

from collections import deque

import numpy as np
import ml_dtypes

import concourse.bass as bass
from concourse import bacc
import concourse.mybir as mybir
import concourse.tile as tile
from concourse.bass_utils import run_bass_kernel_spmd
from concourse.masks import make_identity

F32 = mybir.dt.float32
I16 = mybir.dt.int16
BF16 = mybir.dt.bfloat16
AF = mybir.ActivationFunctionType
ALU = mybir.AluOpType

S = 2048          # sequence length per batch
D = 1024          # model dim
DH = 64           # head dim
NH = 4            # heads per core
GH = NH * DH      # 256 hidden cols per core
KC = D // 128     # 8 contraction chunks of 128
ST = S // 128     # 16 sequence blocks of 128
SPAN = 512        # q-span processed per scores/exp/attnV block
NSP = S // SPAN   # 4 spans
SCALE = DH ** -0.5

# Schraudolph exp on DVE/GpSimd: i16 = trunc(s*C1 + C2); bits viewed as
# bf16 approximate exp(s*SCALE) within ~3.6% max rel err.  2 of every 3
# score tiles go through the approximation (DVE + GpSimd), 1 of 3 through
# exact exp on ACT.
C1 = SCALE * np.log2(np.e) * 128.0
C2 = (127.0 - 0.045) * 128.0 + 0.5

FILL_NS = 1400    # PE filler budget between score groups (ns)

N_CORES = 8


def _build_nc(reps=1):
    nc = bacc.Bacc("TRN2", target_bir_lowering=False)

    qb = nc.declare_dram_parameter("qt", [D, S], BF16, isOutput=False)
    wq = nc.declare_dram_parameter("wq", [D, GH], BF16, isOutput=False)
    wk = nc.declare_dram_parameter("wk", [D, GH], BF16, isOutput=False)
    wv = nc.declare_dram_parameter("wv", [D, GH], BF16, isOutput=False)
    bqk = nc.declare_dram_parameter("bqk", [128, 4], F32, isOutput=False)
    bvb = nc.declare_dram_parameter("bvb", [128, GH], F32, isOutput=False)
    wo = nc.declare_dram_parameter("wo", [GH, D], BF16, isOutput=False)
    out = nc.declare_dram_parameter("partial", [S, D], BF16, isOutput=True)

    with tile.TileContext(nc) as tc:
        with (
            tc.tile_pool(name="wsing", bufs=1) as wsing,
            tc.tile_pool(name="big", bufs=8) as big,
            tc.tile_pool(name="ptp", bufs=6) as ptp,
            tc.tile_pool(name="qk", bufs=6) as qk,
            tc.tile_pool(name="small", bufs=1) as small,
            tc.tile_pool(name="ostage", bufs=4) as ostage,
            tc.tile_pool(name="pmm", bufs=2, space="PSUM") as pmm,
            tc.tile_pool(name="pov", bufs=2, space="PSUM") as pov,
            tc.tile_pool(name="pst", bufs=4, space="PSUM") as pst,
        ):
            # Weight DMAs ride the ACT hwdge / gpsimd swdge queues so the
            # SP queue is free for the qT span chunks the projections need
            # first.
            wk_t = wsing.tile([128, KC, GH], BF16)
            nc.gpsimd.dma_start(out=wk_t, in_=wk[:, :].rearrange("(kc p) f -> p kc f", p=128))
            wq_t = wsing.tile([128, KC, GH], BF16)
            nc.gpsimd.dma_start(out=wq_t, in_=wq[:, :].rearrange("(kc p) f -> p kc f", p=128))
            wv_t = wsing.tile([128, KC, GH], BF16)
            nc.gpsimd.dma_start(out=wv_t, in_=wv[:, :].rearrange("(kc p) f -> p kc f", p=128))
            wo_t = wsing.tile([128, 2, D], BF16)
            nc.gpsimd.dma_start(out=wo_t, in_=wo[:, :].rearrange("(c p) f -> p c f", p=128))
            bqk_t = wsing.tile([128, 4], F32)
            nc.scalar.dma_start(out=bqk_t, in_=bqk[:, :])
            bvb_t = wsing.tile([128, GH], F32)
            nc.scalar.dma_start(out=bvb_t, in_=bvb[:, :])
            # Persistent V' [128, kb, 4 heads x (64 V | 64 ones)]: the ones
            # columns are memset once; attn@V's lhsT M=128 then broadcasts
            # the softmax denominator across PSUM partitions 64-127 free.
            vpr_t = wsing.tile([128, ST, 512], BF16, name="vpr_t")
            nc.vector.memset(
                vpr_t[:, :, :].rearrange("p kb (h c) -> p (kb h) c", c=128)[:, :, 64:128],
                1.0,
            )

            consts = (qb, out, wq_t, wk_t, wv_t, wo_t, bqk_t, bvb_t, vpr_t)
            pools = (big, ptp, qk, small, ostage, pmm, pov, pst)

            def emit_body():
                _emit_body(nc, consts, pools)

            if reps == 1:
                emit_body()
            else:
                # For_i ends each iteration with an all-engine barrier +
                # semaphore reset, so loop iterations cannot overlap.  Unroll
                # the body so body k+1's qT prefetch DMAs overlap body k's
                # compute tail, amortizing the startup/tail latency.
                U = 4 if reps % 4 == 0 else (2 if reps % 2 == 0 else 1)
                with tc.For_i(0, reps // U, 1):
                    for _ in range(U):
                        emit_body()

    nc.compile()
    return nc


def _emit_body(nc, consts, pools):
    (qb, out, wq_t, wk_t, wv_t, wo_t, bqk_t, bvb_t, vpr_t) = consts
    (big, ptp, qk, small, ostage, pmm, pov, pst) = pools

    # ---- filler machinery: FIFO of (span_tag, generator) ----
    fill = deque()

    def push(tag, gen):
        fill.append((tag, gen))

    def run_fill(budget):
        while budget > 0 and fill:
            tag, gen = fill[0]
            try:
                budget -= next(gen)
            except StopIteration:
                fill.popleft()

    def flush_through(tag_max):
        while fill and fill[0][0] <= tag_max:
            tag, gen = fill[0]
            for _ in gen:
                pass
            fill.popleft()

    def flush_all():
        flush_through(1 << 30)

    # ---- qT spans DMA'd directly from DRAM (host pre-transposed) ----
    # Span-major chunk order so the span-0 projections can start after
    # ~1MB of DMA instead of the full 4MB.
    qT = [big.tile([128, S], BF16, tag="big", name=f"qT{j}") for j in range(KC)]

    def prefetch_qt():
        for sp4 in range(4):
            for j in range(KC):
                eng = nc.sync if j % 2 == 0 else nc.scalar
                eng.dma_start(
                    out=qT[j][:, sp4 * 512:(sp4 + 1) * 512],
                    in_=qb[j * 128:(j + 1) * 128, sp4 * 512:(sp4 + 1) * 512],
                )

    # ---- projections ----
    QT = [None, None]
    KT = [None, None]

    def qk_chain(pair, w_t, bias_col, dst, sp4):
        ps = pmm.tile([128, 512], F32, tag="pmm")
        for k in range(KC):
            nc.tensor.matmul(
                ps,
                w_t[:, k, pair * 128:(pair + 1) * 128],
                qT[k][:, sp4 * 512:(sp4 + 1) * 512],
                start=(k == 0),
                stop=(k == KC - 1),
            )
        nc.vector.tensor_scalar_add(
            dst[:, sp4 * 512:(sp4 + 1) * 512],
            ps,
            bqk_t[:, bias_col:bias_col + 1],
        )

    def chain_gen(pair, w_t, bias_col, dst, sp4):
        qk_chain(pair, w_t, bias_col, dst, sp4)
        yield 1700

    def v_gen(blocks):
        for sb in blocks:
            ps = pmm.tile([128, 512], F32, tag="pmm")
            for k in range(KC):
                nc.tensor.matmul(
                    ps[:, :GH],
                    qT[k][:, sb * 128:(sb + 1) * 128],
                    wv_t[:, k, :],
                    start=(k == 0),
                    stop=(k == KC - 1),
                )
            nc.vector.tensor_add(
                vpr_t[:, sb, :].rearrange("p (h c) -> p h c", c=128)[:, :, 0:64],
                ps[:, :GH].rearrange("p (h c) -> p h c", c=64),
                bvb_t.rearrange("p (h c) -> p h c", c=64),
            )
            yield 870

    OT = [qk.tile([128, S], BF16, tag="qk", name=f"OT{c}") for c in range(2)]

    # ---- attention stages ----
    def sx(pair, sp, grp_hooks=None):
        """Scores + exp for one (head-pair, q-span); PE filler between
        groups covers the exp drain of the score PSUM ring.  The exp
        drains alternate ACT (exact exp) / DVE (Schraudolph int16) 1:1 so
        neither engine gates a group's drain."""
        q0 = sp * SPAN
        pt = [ptp.tile([128, ST, SPAN], BF16, tag="pt", name=f"pt{pair}_{sp}_{h}")
              for h in range(2)]
        for grp in range(8):
            ps_st = {}
            for kk in range(2):
                kb = grp * 2 + kk
                for h in range(2):
                    ps_st[kk, h] = pst.tile([128, SPAN], F32, tag="st",
                                            name=f"st{kk}_{h}")
                    nc.tensor.matmul(
                        ps_st[kk, h],
                        KT[pair][h * 64:(h + 1) * 64, kb * 128:(kb + 1) * 128],
                        QT[pair][h * 64:(h + 1) * 64, q0:q0 + SPAN],
                        start=True,
                        stop=True,
                    )
            for kk in range(2):
                for h in range(2):
                    pt_sl = pt[h][:, grp * 2 + kk, :]
                    if (kk + h) % 2 == 1:
                        nc.vector.tensor_scalar(
                            pt_sl.bitcast(I16), ps_st[kk, h], C1, C2,
                            ALU.mult, ALU.add,
                        )
                    else:
                        nc.scalar.activation(pt_sl, ps_st[kk, h], AF.Exp,
                                             scale=SCALE)
            if grp_hooks and grp in grp_hooks:
                grp_hooks[grp]()
            else:
                run_fill(FILL_NS)
        return pt

    def av_gen(pair, sp, pt):
        """attn@V + normalize, as ~0.9 µs units."""
        for h in range(2):
            hh = pair * 2 + h
            ov = pov.tile([128, SPAN], F32, tag="ov")
            for kq in range(4):
                for kb in range(kq * 4, kq * 4 + 4):
                    nc.tensor.matmul(
                        ov,
                        vpr_t[:, kb, hh * 128:(hh + 1) * 128],
                        pt[h][:, kb, :],
                        start=(kb == 0),
                        stop=(kb == ST - 1),
                    )
                yield 870
            rdb = small.tile([64, SPAN], F32, tag="rdb")
            nc.vector.reciprocal(rdb, ov[64:128, :])
            nc.vector.tensor_mul(
                OT[pair][h * 64:(h + 1) * 64, sp * SPAN:(sp + 1) * SPAN],
                ov[0:DH, :],
                rdb,
            )
            yield 400

    def outproj_gen(sbs, copy_on_act=False):
        for sb in sbs:
            o_tile = ostage.tile([128, D], BF16, tag="ostage")
            for n in range(2):
                ps = pmm.tile([128, 512], F32, tag="pmm")
                for c in range(2):
                    nc.tensor.matmul(
                        ps,
                        OT[c][:, sb * 128:(sb + 1) * 128],
                        wo_t[:, c, n * 512:(n + 1) * 512],
                        start=(c == 0),
                        stop=(c == 1),
                    )
                if copy_on_act:
                    nc.scalar.copy(o_tile[:, n * 512:(n + 1) * 512], ps)
                else:
                    nc.vector.tensor_copy(o_tile[:, n * 512:(n + 1) * 512], ps)
            nc.gpsimd.dma_start(out=out[sb * 128:(sb + 1) * 128, :], in_=o_tile)
            yield 900

    # ---- emission: score bursts with 128x128-mode filler blocks between ----
    def proj_tile(pair, name):
        t = qk.tile([128, S], BF16, tag="qk", name=name)
        return t

    KT[0] = proj_tile(0, "ktp0")
    QT[0] = proj_tile(0, "qtp0")
    KT[1] = proj_tile(1, "ktp1")
    QT[1] = proj_tile(1, "qtp1")
    prefetch_qt()
    qk_chain(0, wk_t, 2, KT[0], 0)
    qk_chain(0, wq_t, 0, QT[0], 0)

    def pro(tg):
        def fn():
            qk_chain(0, wk_t, 2, KT[0], tg)
        return fn

    # Filler supply for the pair-0 spans: V projection, then pair-1 K/Q.
    push(0, v_gen(range(ST)))
    for sp4 in range(4):
        push(1, chain_gen(1, wk_t, 3, KT[1], sp4))
    for sp4 in range(4):
        push(1, chain_gen(1, wq_t, 1, QT[1], sp4))

    order = [(0, sp) for sp in range(NSP)] + [(1, sp) for sp in range(NSP)]
    for idx, (pair, sp) in enumerate(order):
        if pair == 0 and sp > 0:
            qk_chain(0, wq_t, 0, QT[0], sp)
        flush_through(idx - 3)
        hooks = {1: pro(1), 3: pro(2), 5: pro(3)} if idx == 0 else None
        pt = sx(pair, sp, grp_hooks=hooks)
        push(idx, av_gen(pair, sp, pt))
        if pair == 1:
            push(idx, outproj_gen(range(sp * 4, sp * 4 + 4), copy_on_act=(sp >= 2)))
    flush_all()


_NC_CACHE = {}


def _get_nc(reps=1):
    if reps not in _NC_CACHE:
        _NC_CACHE[reps] = _build_nc(reps)
    return _NC_CACHE[reps]


def _shard_inputs(q, Wq, bq, Wkv, bkv, Wo, bo):
    q = np.asarray(q, dtype=np.float32)
    Wq = np.asarray(Wq, dtype=np.float32)
    bq = np.asarray(bq, dtype=np.float32)
    Wkv = np.asarray(Wkv, dtype=np.float32)
    bkv = np.asarray(bkv, dtype=np.float32)
    Wo = np.asarray(Wo, dtype=np.float32)

    HID = D  # 1024 total hidden
    in_maps = []
    for c in range(N_CORES):
        b, g = divmod(c, 4)
        lo = g * GH
        wk_s = Wkv[:, lo:lo + GH]
        wv_s = Wkv[:, HID + lo:HID + lo + GH]
        bq_s = bq[lo:lo + GH]
        bk_s = bkv[lo:lo + GH]
        bv_s = bkv[HID + lo:HID + lo + GH]

        bqk_pp = np.stack(
            [bq_s[0:128], bq_s[128:256], bk_s[0:128], bk_s[128:256]], axis=1
        )

        in_maps.append({
            "qt": np.ascontiguousarray(q[b].T).astype(ml_dtypes.bfloat16),
            "wq": np.ascontiguousarray(Wq[:, lo:lo + GH]).astype(ml_dtypes.bfloat16),
            "wk": np.ascontiguousarray(wk_s).astype(ml_dtypes.bfloat16),
            "wv": np.ascontiguousarray(wv_s).astype(ml_dtypes.bfloat16),
            "bqk": np.ascontiguousarray(bqk_pp),
            "bvb": np.broadcast_to(bv_s, (128, GH)).copy(),
            "wo": np.ascontiguousarray(Wo[lo:lo + GH, :]).astype(ml_dtypes.bfloat16),
        })
    return in_maps


def _gather(results, bo):
    bo = np.asarray(bo, dtype=np.float32)
    out = np.empty((2, S, D), dtype=np.float32)
    for b in range(2):
        acc = results[4 * b]["partial"].astype(np.float32)
        for g in range(1, 4):
            acc = acc + results[4 * b + g]["partial"].astype(np.float32)
        out[b] = acc + bo
    return out


_RUNNER_CACHE = {}


def _make_runner(reps=1):
    """Build (once) a reusable jitted SPMD callable for the given rep count.

    Re-jitting per call loads a second copy of the NEFF and has been seen to
    wedge the exec unit, so the jitted executable is cached per process.
    """
    if reps in _RUNNER_CACHE:
        return _RUNNER_CACHE[reps]

    import jax
    from jax.sharding import Mesh, PartitionSpec
    from jax.experimental.shard_map import shard_map
    from concourse import bass2jax

    nc = _get_nc(reps)
    bass2jax.install_neuronx_cc_hook()
    partition_name = nc.partition_id_tensor.name if nc.partition_id_tensor else None
    in_names, out_names, out_avals, zero_outs = [], [], [], []
    for alloc in nc.m.functions[0].allocations:
        if not isinstance(alloc, mybir.MemoryLocationSet):
            continue
        name = alloc.memorylocations[0].name
        if alloc.kind == "ExternalInput":
            if name != partition_name:
                in_names.append(name)
        elif alloc.kind == "ExternalOutput":
            out_names.append(name)
            shape = tuple(alloc.tensor_shape)
            dtype = mybir.dt.np(alloc.dtype)
            out_avals.append(jax.core.ShapedArray(shape, dtype))
            zero_outs.append(np.zeros(shape, dtype))
    n_params = len(in_names)
    n_outs = len(out_avals)
    in_names.extend(out_names)
    if partition_name:
        in_names.append(partition_name)

    def _body(*args):
        operands = list(args)
        if partition_name:
            operands.append(bass2jax.partition_id_tensor())
        return tuple(bass2jax._bass_exec_p.bind(
            *operands,
            out_avals=tuple(out_avals),
            in_names=tuple(in_names),
            out_names=tuple(out_names),
            lowering_input_output_aliases=(),
            sim_require_finite=True,
            sim_require_nnan=True,
            nc=nc,
        ))

    devices = jax.devices()[:N_CORES]
    mesh = Mesh(np.asarray(devices), ("core",))
    donate = tuple(range(n_params, n_params + n_outs))
    sharded = jax.jit(
        shard_map(_body, mesh=mesh,
                  in_specs=(PartitionSpec("core"),) * (n_params + n_outs),
                  out_specs=(PartitionSpec("core"),) * len(out_names),
                  check_rep=False),
        donate_argnums=donate, keep_unused=True)

    def run(in_maps):
        per_core = [[np.asarray(m[nm]) for nm in in_names[:n_params]]
                    for m in in_maps]
        concat_in = [np.concatenate([per_core[c][i] for c in range(N_CORES)],
                                    axis=0) for i in range(n_params)]
        zo = [np.concatenate([z] * N_CORES, axis=0) for z in zero_outs]
        outs = sharded(*concat_in, *zo)
        outs = [np.asarray(o) for o in outs]
        per_core_res = []
        for c in range(N_CORES):
            per_core_res.append({
                name: np.split(outs[i], N_CORES, axis=0)[c]
                for i, name in enumerate(out_names)
            })
        return per_core_res

    _RUNNER_CACHE[reps] = run
    return run


def _run(inputs, reps=1):
    run = _make_runner(reps)
    in_maps = _shard_inputs(**inputs)
    results = run(in_maps)
    out = _gather(results, inputs["bo"])
    return out, results


def kernel(q, Wq, bq, Wkv, bkv, Wo, bo):
    out, _ = _run(dict(q=q, Wq=Wq, bq=bq, Wkv=Wkv, bkv=bkv, Wo=Wo, bo=bo))
    return out

